# revision 9
# baseline (speedup 1.0000x reference)
"""Trainium2 Bass kernel for an attention block (B=4, C=64, H=W=64).

reference:
    xf = x.reshape(B, C, N)                      # N = H*W = 4096
    qkv = w_qkv @ xf + b_qkv                     # [B, 3C, N]
    q, k, v = split(qkv)
    attn = softmax(q^T k / sqrt(C), axis=-1)     # [B, N, N]
    out = w_proj @ (v @ attn^T) + b_proj + x

Sharding: 8 cores = (batch sample, query half). Each core receives its
sample's tokens ROTATED so its own 2048 queries are columns 0:2048
(attention is permutation-invariant over keys). No collectives.

Numerics: the q/k projections fold into A = Wk^T Wq on the host (exact);
scores[k, q] = x_k . (A x_q + c). Both score operands are fp8e4m3 and the
score matmuls run in DoubleRow perf mode (2 fp8 contraction elements per
PE cell -> 0.5 cycles/output column). x is quantized to fp8 on the host
(dtype/layout prep only, like the baseline's bf16 weight prep); qw = A
x_q + c is computed on-device in f32 then quantized, with a constant +20
row appended through the projection weights so the fp8-bit-trick exp
needs no add instruction. softmax uses exp(s/8)*const (the const cancels
between numerator and denominator): ACT computes it exactly via the Exp
table; DVE produces the fp8e4m3 BIT PATTERN directly as
round(max(s_psum,0)/ln2) (Schraudolph) through an int8 view. E is fp8
and V is fp8 (output projection folded on the host), so the AV matmul is
also DoubleRow fp8, with a ones column appended to V^T producing the
softmax denominator for free. The reciprocal row is broadcast across
partitions with a contraction-dim-1 matmul; b_eff = w_proj@b_v + b_proj
folds both remaining biases since softmax rows sum to 1.

Pipeline: scores psum tiles are [128 keys, 1024 queries] (2 banks, ring
of 3) so three key-chunks of exp are in flight across ACT and DVE (the
only engines that may read PSUM — GPSIMD cannot touch it on real HW, so
it only gets the SBUF-side bias+residual adds, memsets and DMA issue).
The qw / V-projection prep and the epilogue broadcast borrow scores-ring
turns instead of their own banks; the AV accumulator is split into two
independent per-half [65,512] banks so the epilogue's reciprocal can
start as soon as its own half's accumulation group closes.
"""

import numpy as np
import ml_dtypes

import concourse.bass as bass
import concourse.tile as tile
from concourse import mybir
from concourse.bass_utils import run_bass_kernel_spmd

B, C = 4, 64
N = 4096          # H*W tokens (keys per core)
QH = N // 2       # queries per core
QB = 1024         # query block
NQB = QH // QB
MC = 128          # key chunk = scores partition dim
NMC = N // MC     # 32
NPAIR = NMC // 2  # DoubleRow AV key-pairs

_F32 = mybir.dt.float32
_F32R = mybir.dt.float32r
_FP8 = mybir.dt.float8e4
_I8 = mybir.dt.int8
_EXP = mybir.ActivationFunctionType.Exp
_DR = mybir.MatmulPerfMode.DoubleRow
_ADD = mybir.AluOpType.add
_MAX = mybir.AluOpType.max
_MULT = mybir.AluOpType.mult

_LN2 = 0.6931471805599453
SCH_A = 1.0 / _LN2                 # fp8 bits per unit psum-score
SBIAS = 20.0                       # +20 folded into scores via const row
BETA_ACT = -0.125 * SBIAS + 2.5 - 56.0 * _LN2 / 8.0   # = -4.852

# exp-engine schedule: one char per key-chunk; 'a'=ACT exact exp (1038ns),
# 'd'=DVE schraudolph (1262ns). Greedy least-finish-time with each
# engine's fixed auxiliary load (warm+copies vs copies+recips+muls).
def _mk_pattern():
    import os

    spec = os.environ.get("KV2_PAT", "4000,5200")
    la, ld = spec.split(",")
    cost = {"a": 1038.0, "d": 1262.0}
    load = {"a": float(la), "d": float(ld)}
    out = []
    for _ in range(64):
        e = min("ad", key=lambda k: load[k] + cost[k])
        load[e] += cost[e]
        out.append(e)
    s = "".join(out)
    return (s[:32], s[32:])


_PAT = _mk_pattern()


def _split_excess_waits(nc):
    """walrus CoreV3 in this toolchain accepts at most one sync wait per
    instruction; move extras onto NoOps spliced just before it."""
    for f in nc.m.functions:
        for bb in f.blocks:
            new_insts = []
            changed = False
            for inst in bb.instructions:
                si = inst.sync_info
                if si is not None and si.on_wait and len(si.on_wait) > 1:
                    waits = list(si.on_wait)
                    extra, keep = waits[:-1], waits[-1:]
                    for w in extra:
                        nop = mybir.InstNoOp(name=nc.get_next_instruction_name())
                        nop.engine = inst.engine
                        nop.sync_info = mybir.SyncInfo(on_wait=[w], on_update=[])
                        nc.register_instruction(nop)
                        new_insts.append(nop)
                    si.on_wait = keep
                    changed = True
                new_insts.append(inst)
            if changed:
                bb.instructions = new_insts


def _r(ap):
    return ap.bitcast(_F32R)


def build_graph():
    nc = bass.Bass("TRN2", target_bir_lowering=False, debug=False)

    x_ext = nc.declare_dram_parameter("x", [C + 1, QH], _F32, isOutput=False)
    x8_ext = nc.declare_dram_parameter("x8", [33, 2, N], _I8, isOutput=False)
    aqt_ext = nc.declare_dram_parameter("aqt", [C + 1, 66], _F32, isOutput=False)
    w8vp_ext = nc.declare_dram_parameter("w8vp", [33, 2, C], _I8, isOutput=False)
    beff_ext = nc.declare_dram_parameter("beff", [C, 1], _F32, isOutput=False)
    ones1_ext = nc.declare_dram_parameter("ones1", [1, C], _F32, isOutput=False)
    out_ext = nc.declare_dram_parameter("out", [C, QH], _F32, isOutput=True)

    with (
        nc.allow_low_precision(reason="fp8 attention within tolerance"),
        tile.TileContext(nc) as tc,
        tc.tile_pool(name="consts", bufs=1) as consts,
        # PSUM (8 banks): scores ring 3x[128,1024]=6 banks; prep (qw/v-proj)
        # and the epilogue broadcast borrow ring turns. AV accumulator:
        # 2x[65,512] = 2 banks.
        tc.tile_pool(name="spool", bufs=3, space="PSUM") as spool,
        tc.tile_pool(name="avpool", bufs=1, space="PSUM") as avpool,
        tc.tile_pool(name="ebuf", bufs=4) as ebuf,
        tc.tile_pool(name="obuf", bufs=6) as obuf,
    ):
        X = consts.tile([C + 1, QH], _F32R, tag="x")
        X8 = consts.tile([33, 2, N], _FP8, tag="x8")
        AQT = consts.tile([C + 1, 66], _F32R, tag="aqt")
        W8 = consts.tile([33, 2, C], _FP8, tag="w8")
        QW8 = consts.tile([33, 2, QH], _FP8, tag="qw8")
        VT8 = consts.tile([MC, NPAIR, 2, C + 1], _FP8, tag="vt8")
        BEFF = consts.tile([C, 1], _F32, tag="beff")
        ONES1 = consts.tile([1, C], _F32R, tag="ones1")

        # ---- input DMAs; first PE op needs aqt + x cols 0:512 ----
        nc.sync.dma_start(out=AQT, in_=_r(aqt_ext[:, :]))
        nc.sync.dma_start(out=X[:, 0:512], in_=_r(x_ext[:, 0:512]))
        nc.gpsimd.dma_start(out=X8[:, :, 0:1024].bitcast(_I8), in_=x8_ext[:, :, 0:1024])
        nc.sync.dma_start(out=X[:, 512:1024], in_=_r(x_ext[:, 512:1024]))
        nc.gpsimd.dma_start(out=W8.bitcast(_I8), in_=w8vp_ext[:, :, :])
        nc.sync.dma_start(out=X8[:, :, 1024:2048].bitcast(_I8), in_=x8_ext[:, :, 1024:2048])
        nc.sync.dma_start(out=X[:, 1024:1536], in_=_r(x_ext[:, 1024:1536]))
        nc.sync.dma_start(out=X8[:, :, 2048:3072].bitcast(_I8), in_=x8_ext[:, :, 2048:3072])
        nc.sync.dma_start(out=X[:, 1536:2048], in_=_r(x_ext[:, 1536:2048]))
        nc.sync.dma_start(out=X8[:, :, 3072:4096].bitcast(_I8), in_=x8_ext[:, :, 3072:4096])
        nc.sync.dma_start(out=BEFF, in_=beff_ext[:, :])
        nc.sync.dma_start(out=ONES1, in_=_r(ones1_ext[:, :]))

        # V^T ones column (softmax denominator row) + exp table preload
        nc.gpsimd.memset(VT8[:, :, :, C : C + 1], 1.0)
        BACT = consts.tile([MC, 1], _F32, tag="bact")
        nc.gpsimd.memset(BACT, BETA_ACT)
        WARM = consts.tile([1, 1], _F32, tag="warm")
        nc.vector.memset(WARM, 0.0)
        nc.scalar.activation(WARM, WARM, _EXP, bias=0.0, scale=1.0)

        def act_copy(out, in_):
            nc.scalar.copy(out, in_)

        def dve_copy(out, in_):
            nc.vector.tensor_copy(out, in_)

        # ---- prep emitters (psum borrowed from the scores ring) ----
        def emit_pq(j, copy_eng):
            # qw channel-halves a=0/1 (incl. the +20 const row for a=0 /
            # zero row for a=1) for queries j*512:(j+1)*512, then one fp8
            # quantize-copy into QW8
            lo = j * 512
            ps = spool.tile([33, QB], _F32, tag="s", name="pq")
            for a in (0, 1):
                nc.tensor.matmul(
                    ps[:, a * 512 : (a + 1) * 512],
                    AQT[:, a * 33 : (a + 1) * 33],
                    X[:, lo : lo + 512],
                    start=True,
                    stop=True,
                )
            src = ps.rearrange("p (i q) -> p i q", i=2)
            copy_eng(QW8[0:33, :, lo : lo + 512], src)

        def emit_pv(g, copy_eng):
            # projected V^T for key chunks 16g..16g+15 (fp8 DoubleRow, out
            # free 64 -> 32 cycles each), then one fp8 quantize-copy
            ps = spool.tile([MC, QB], _F32, tag="s", name="pv")
            for j in range(16):
                mc = 16 * g + j
                nc.tensor.matmul(
                    ps[:, j * C : (j + 1) * C],
                    X8[:, :, mc * MC : (mc + 1) * MC],
                    W8,
                    start=True,
                    stop=True,
                    perf_mode=_DR,
                )
            src = ps.rearrange("p (t i c) -> p t i c", t=8, i=2)
            copy_eng(VT8[:, 8 * g : 8 * g + 8, :, 0:C], src)

        # ---- exp over one key chunk (both query halves, one op) ----
        def emit_exp(eng, dst, src):
            src = src.rearrange("p (h q) -> p h q", h=2)
            if eng == "a":
                nc.scalar.activation(dst, src, _EXP, bias=BACT, scale=0.125)
            else:
                nc.vector.tensor_scalar(
                    out=dst.bitcast(_I8),
                    in0=src,
                    scalar1=0.0,
                    scalar2=SCH_A,
                    op0=_MAX,
                    op1=_MULT,
                )

        def emit_av(t, h, pav, E8t):
            nc.tensor.matmul(
                pav[h // 512],
                VT8[:, t, :, :],
                E8t[:, h // 512, :, :],
                start=(t == 0),
                stop=(t == NPAIR - 1),
                perf_mode=_DR,
            )

        def epi_stage1(qb, pav, uts):
            # drain the finished accumulator halves to SBUF (a DVE/ACT op
            # may read only ONE psum operand, so the mul below needs U in
            # SBUF while the broadcast stays in psum)
            for h in (0, 512):
                U = obuf.tile([C + 1, 512], _F32, tag="u")
                (act_copy if h == 0 else dve_copy)(U, pav[h // 512])
                uts.append(U)

        def epi_stage2(qb, uts, r1s, pbs):
            for h in (0, 512):
                R1 = obuf.tile([1, 512], _F32R, tag="r1")
                nc.vector.reciprocal(R1, uts[h // 512][C : C + 1, :])
                r1s.append(R1)
            for h in (0, 512):
                pb = spool.tile([C, 512], _F32, tag="s", name="pb")
                nc.tensor.matmul(pb, ONES1, r1s[h // 512], start=True, stop=True)
                pbs.append(pb)

        def epi_stage3(qb, uts, pbs):
            q0 = qb * QB
            for h in (0, 512):
                i = h // 512
                UN = obuf.tile([C, 512], _F32, tag="un")
                nc.vector.tensor_mul(UN, uts[i][0:C, :], pbs[i])
                O = obuf.tile([C, 512], _F32, tag="o")
                nc.gpsimd.scalar_tensor_tensor(
                    out=O,
                    in0=UN,
                    scalar=BEFF,
                    in1=X[0:C, q0 + h : q0 + h + 512],
                    op0=_ADD,
                    op1=_ADD,
                )
                nc.sync.dma_start(out=out_ext[:, q0 + h : q0 + h + 512], in_=O)

        # ---- main pipeline ----
        emit_pq(0, act_copy)
        emit_pq(1, dve_copy)
        emit_pv(0, act_copy)
        prep = {
            1: (emit_pv, (1, dve_copy)),
            3: (emit_pq, (2, act_copy)),
            5: (emit_pq, (3, dve_copy)),
        }

        pend = None     # previous block's (qb, pav); epilogue staged here
        pend_av = None  # previous block's deferred last AV pairs
        uts, r1s, pbs = [], [], []
        for qb in range(NQB):
            q0 = qb * QB
            pav0 = avpool.tile([C + 1, 512], _F32, tag="av", name="pav0", bufs=2)
            pav1 = avpool.tile([C + 1, 512], _F32, tag="av", name="pav1", bufs=2)
            pav = (pav0, pav1)
            etiles = {}
            for m in range(NMC):
                ps = spool.tile([MC, QB], _F32, tag="s", name="ps")
                for h in (0, 512):
                    nc.tensor.matmul(
                        ps[:, h : h + 512],
                        X8[:, :, m * MC : (m + 1) * MC],
                        QW8[:, :, q0 + h : q0 + h + 512],
                        start=True,
                        stop=True,
                        perf_mode=_DR,
                    )
                if pend is not None:
                    # previous block's last AV pairs + epilogue ride behind
                    # this block's first chunks in every engine queue
                    if m == 1:
                        emit_av(NPAIR - 2, 0, pend[1], pend_av[0])
                        emit_av(NPAIR - 2, 512, pend[1], pend_av[0])
                    elif m == 2:
                        emit_av(NPAIR - 1, 0, pend[1], pend_av[1])
                        emit_av(NPAIR - 1, 512, pend[1], pend_av[1])
                        epi_stage1(pend[0], pend[1], uts)
                    elif m == 3:
                        epi_stage2(pend[0], uts, r1s, pbs)
                    elif m == 4:
                        epi_stage3(pend[0], uts, pbs)
                        pend, uts, r1s, pbs = None, [], [], []
                if m % 2 == 0:
                    E8t = ebuf.tile([MC, 2, 2, 512], _FP8, tag="e", name="e8t")
                    etiles[m // 2] = E8t
                emit_exp(_PAT[qb][m], etiles[m // 2][:, :, m % 2, :], ps)
                if qb == 0 and m in prep:
                    fn, args = prep[m]
                    fn(*args)
                if m >= 6 and m % 2 == 0:
                    t = (m - 6) // 2
                    emit_av(t, 0, pav, etiles[t])
                    emit_av(t, 512, pav, etiles[t])
            emit_av(NPAIR - 3, 0, pav, etiles[NPAIR - 3])
            emit_av(NPAIR - 3, 512, pav, etiles[NPAIR - 3])
            pend = (qb, pav)
            pend_av = (etiles[NPAIR - 2], etiles[NPAIR - 1])
        # final tail: last AV pairs per half, then a stage-major epilogue so
        # the recip -> broadcast -> mul -> add -> dma chain pipelines
        qb, pav = pend
        q0 = qb * QB
        for t in (NPAIR - 2, NPAIR - 1):
            emit_av(t, 0, pav, pend_av[t - NPAIR + 2])
            emit_av(t, 512, pav, pend_av[t - NPAIR + 2])
        utf, r1f, unf = [], [], []
        for i in (0, 1):
            U = obuf.tile([C + 1, 512], _F32, tag="uf")
            (act_copy if i == 0 else dve_copy)(U, pav[i])
            utf.append(U)
        for i in (0, 1):
            R1 = obuf.tile([1, 512], _F32R, tag="r1f")
            nc.vector.reciprocal(R1, utf[i][C : C + 1, :])
            r1f.append(R1)
        pbf = []
        for i in (0, 1):
            pb = spool.tile([C, 512], _F32, tag="s", name="pbf")
            nc.tensor.matmul(pb, ONES1, r1f[i], start=True, stop=True)
            pbf.append(pb)
        for i in (0, 1):
            UN = obuf.tile([C, 512], _F32, tag="unf")
            nc.vector.tensor_mul(UN, utf[i][0:C, :], pbf[i])
            unf.append(UN)
        for k in range(4):
            i, o = k // 2, (k % 2) * 256
            O = obuf.tile([C, 256], _F32, tag="of")
            se = nc.gpsimd if k % 2 == 0 else nc.vector
            se.scalar_tensor_tensor(
                out=O,
                in0=unf[i][:, o : o + 256],
                scalar=BEFF,
                in1=X[0:C, q0 + i * 512 + o : q0 + i * 512 + o + 256],
                op0=_ADD,
                op1=_ADD,
            )
            eng = nc.sync if k % 2 == 0 else nc.scalar
            eng.dma_start(
                out=out_ext[:, q0 + i * 512 + o : q0 + i * 512 + o + 256], in_=O
            )

    _split_excess_waits(nc)
    return nc


_GRAPH_CACHE = {}


def _get_graph():
    if "nc" not in _GRAPH_CACHE:
        _GRAPH_CACHE["nc"] = build_graph()
    return _GRAPH_CACHE["nc"]


_E4 = ml_dtypes.float8_e4m3


def _fp8bits(a):
    return np.asarray(a, dtype=np.float32).astype(_E4).view(np.int8)


def make_in_maps(x, w_qkv, b_qkv, w_proj, b_proj):
    xf = np.ascontiguousarray(np.asarray(x, dtype=np.float32).reshape(B, C, N))
    w_qkv = np.asarray(w_qkv, dtype=np.float32)
    b_qkv = np.asarray(b_qkv, dtype=np.float32)
    w_proj = np.asarray(w_proj, dtype=np.float32)
    b_proj = np.asarray(b_proj, dtype=np.float32)

    # scores = x_k . (A x_q + c): A = Wk^T Wq, c = Wk^T b_q (weight prep)
    A = (w_qkv[C : 2 * C].T @ w_qkv[0:C]).astype(np.float32)
    c = (w_qkv[C : 2 * C].T @ b_qkv[0:C]).astype(np.float32)
    # aqt: lhsT halves [65, 33] side by side: rows 0:64 = A-half^T, row 64
    # = c-half; col 32 of each half is the score-bias row (+20 / 0)
    aqt = np.zeros((C + 1, 66), dtype=np.float32)
    for a in range(2):
        aqt[0:C, a * 33 : a * 33 + 32] = A[a * 32 : (a + 1) * 32, :].T
        aqt[C, a * 33 : a * 33 + 32] = c[a * 32 : (a + 1) * 32]
    aqt[C, 32] = SBIAS   # qw row 32: +20 const (half 0) / 0 (half 1)
    # fold the output projection into the v projection (weight prep)
    w_vpT = np.ascontiguousarray((w_proj @ w_qkv[2 * C :]).T.astype(np.float32))
    w8vp = np.zeros((33, 2, C), dtype=np.int8)
    for i in range(2):
        w8vp[0:32, i, :] = _fp8bits(w_vpT[i * 32 : (i + 1) * 32, :])
    b_eff = (w_proj @ b_qkv[2 * C :] + b_proj).reshape(C, 1).astype(np.float32)
    ones1 = np.ones((1, C), dtype=np.float32)

    in_maps = []
    for core in range(8):
        b, h = divmod(core, 2)
        # rotate tokens so this core's queries are columns 0:QH
        xr = np.ascontiguousarray(np.roll(xf[b], -h * QH, axis=1))
        xq = np.empty((C + 1, QH), dtype=np.float32)
        xq[0:C] = xr[:, 0:QH]
        xq[C] = 1.0
        # fp8 DoubleRow layout of x over all keys (dtype/layout prep only)
        x8 = np.zeros((33, 2, N), dtype=np.int8)
        xr8 = _fp8bits(xr)
        for i in range(2):
            x8[0:32, i, :] = xr8[i * 32 : (i + 1) * 32, :]
        x8[32, 0, :] = _fp8bits(np.full(N, 1.0))
        in_maps.append(
            {
                "x": xq,
                "x8": x8,
                "aqt": aqt,
                "w8vp": w8vp,
                "beff": b_eff,
                "ones1": ones1,
            }
        )
    return in_maps


def kernel(x, w_qkv, b_qkv, w_proj, b_proj):
    x = np.asarray(x)
    nc = _get_graph()
    in_maps = make_in_maps(x, w_qkv, b_qkv, w_proj, b_proj)
    res = run_bass_kernel_spmd(nc, in_maps, core_ids=list(range(8)))
    out = np.empty((B, C, N), dtype=np.float32)
    for core in range(8):
        b, h = divmod(core, 2)
        out[b][:, h * QH : (h + 1) * QH] = res.results[core]["out"]
    return out.reshape(x.shape).astype(np.float32)
